# revision 53
# baseline (speedup 1.0000x reference)
"""Trainium2 Bass kernel: 16-head attention block (B=2, S=2048, H=1024).

Sharding: 8 cores = 2-way data parallel (batch) x 4-way tensor parallel
(head groups of 4 heads / 256 dims). Each core computes, for its batch
and head group:
    Q^T, K^T (= W @ x^T, [dims, seq] layout; Wq/bq pre-scaled by 1/8 on
    host so no score scaling is needed on device), V ([seq, dims]),
    S^T = K Q^T per head (key positions on partitions),
    P^T = exp(S^T + mask),
    ctx'^T = [V | 1]^T P^T    (ones column folded in -> row 64 = softmax
                               denominator),
    ctx^T normalized, then partial output O_g = ctx^T.T @ Wo[:,hs]^T.
Host sums the 4 partial outputs per batch and adds bo.

Precision: x and the QKV weights ship as bf16 (halves the inbound DMA);
scores run as float32r; the probabilities P and V are bf16 for the ctx
matmul; the output projection is float32r.  3/8 of the softmax exp
half-tiles run as an int16 Schraudolph (bitcast) on the otherwise-
saturated DVE instead of the ACT engine.  Measured end-to-end rel err
~1.0e-2 against the fp32 reference (gate: 2e-2).

Overlap: emission order interleaves pair-1 projections under pair-0's
softmax shadow and the output projection under pair-1's; per-head score
psum tiles + a 6-deep pt pool keep the PE's score->exp->ctx pipeline
from stalling on any single engine's queue.
"""

import contextlib

import numpy as np

import concourse.bass as bass
import concourse.mybir as mybir
import concourse.tile as tile
from concourse import bacc
from concourse.bass_utils import run_bass_kernel_spmd

B, S, H = 2, 2048, 1024
NUM_HEADS, HEAD_DIM = 16, 64
N_CORES = 8
GROUPS = 4                  # head-parallel groups per batch
HD = H // GROUPS            # 256 head-dims per core (4 heads)
P = 128
KT_H = H // P               # 8 k-tiles over hidden dim
KT_S = S // P               # 16 k-tiles over sequence (key positions)
NCH = 4                     # q chunks
CHUNK = S // NCH            # 512
F32 = mybir.dt.float32
F32R = mybir.dt.float32r
BF16 = mybir.dt.bfloat16
I16 = mybir.dt.int16
EXP = mybir.ActivationFunctionType.Exp

# exp(s) ~= bitcast_bf16(int16(s * 2^7/ln2 + (127*2^7 - c))).  c tuned for
# min-rms relative ripple of the linear-mantissa approximation (~1.5% rms).
# The DVE/Pool f32->i16 convert is round-to-nearest (verified on device);
# softmax's ratio cancels the uniform component, leaving only the ripple.
SCH_A = float(128.0 / np.log(2.0))
SCH_B = float(127.0 * 128.0 - 0.043677448 * 128.0)
# Per-(kt, head-half) exp engine: 'A' = ACT (exact exp), 'V' = DVE
# (int16 Schraudolph).  The Pool engine cannot touch PSUM on TRN2, so it
# only carries SBUF->SBUF traffic (the ctxT bounce copies).  ACT keeps
# 20/32 half-tiles; the DVE share is sized so neither queue bursts past
# the PE's per-kt pace.
def _exp_sched(p):
    sched = {}
    dve_o = (1, 3, 5, 7, 9, 11, 13, 15)
    dve_e = (2, 6, 10, 14)
    for kt in range(KT_S):
        sched[(kt, 0)] = 'V' if kt in dve_e else 'A'
        sched[(kt, 1)] = 'V' if kt in dve_o else 'A'
    return sched


EXP_SCHED = (_exp_sched(0), _exp_sched(1))

_PROGRAM_CACHE = {}


def _emit(tc, nc, dram, masked, with_bias):
    mm = nc.tensor.matmul
    xT_d, wq_d, wk_d, wv_d, wo_d, bq_d, bk_d, bv_d, am_d, o_d = dram

    stack = contextlib.ExitStack()
    with stack:
        const = stack.enter_context(tc.tile_pool(name="const", bufs=1))
        big = stack.enter_context(tc.tile_pool(name="big", bufs=1))

        onesf = const.tile([P, 64], F32)
        nc.any.memset(onesf[:], 1.0)
        ones64 = const.tile([P, 64], F32R)   # lane-64 row used as K=1 lhsT
        nc.vector.tensor_copy(ones64[:], onesf[:])
        onesb = const.tile([P, 64], BF16)
        nc.vector.tensor_copy(onesb[:], onesf[:])
        # warm the ACT exp table before it is first needed
        trash = const.tile([1, 16], F32)
        nc.scalar.activation(trash[:], onesf[0:1, 0:16], EXP)
        if masked:
            amask_sb = const.tile([P, KT_S], F32)
            nc.sync.dma_start(out=amask_sb[:], in_=am_d[:])
        if with_bias:
            ones_sb = const.tile([1, 512], F32R)
            for i in range(8):
                nc.vector.tensor_copy(ones_sb[0:1, i * 64:(i + 1) * 64],
                                      onesf[0:1, :])
            bq_sb = const.tile([1, HD], F32R)
            nc.sync.dma_start(out=bq_sb[:], in_=bq_d[:])
            bk_sb = const.tile([1, HD], F32R)
            nc.sync.dma_start(out=bk_sb[:], in_=bk_d[:])
            bv_sb = const.tile([1, HD], F32R)
            nc.sync.dma_start(out=bv_sb[:], in_=bv_d[:])
        wo_sb = const.tile([P, HD // P, H], F32R)

        # persistent activations
        qT_sb = big.tile([P, 2, S], F32R)    # [dim-in-pair, pair, seq]
        kT_sb = big.tile([P, 2, S], F32R)
        v_sb = big.tile([P, KT_S, GROUPS, HEAD_DIM + 1], BF16)  # [seq, kt, head, d+1]
        ctxT_sb = big.tile([P, 2, S], F32R)

        # ones column of V' (the rowsum trick)
        nc.vector.tensor_copy(v_sb[:, :, :, HEAD_DIM:HEAD_DIM + 1], onesb[:, 0:KT_S * GROUPS])

        # ---------- input DMAs, interleaved k-tile-wise so the pair-0 K/Q
        # projection for seq-chunk 0 can start ~0.5MB into the inbound
        # stream instead of after all of wk+wq (4µs vs 14µs) ----------
        kq_pool = tc.alloc_tile_pool(name="kq_pool", bufs=1, side="right")
        wv_stack = contextlib.ExitStack()
        wv_pool = wv_stack.enter_context(tc.tile_pool(name="wv_pool", bufs=1, side="right"))
        wk_sb = kq_pool.tile([P, KT_H, HD], BF16)
        xT_sb = kq_pool.tile([P, KT_H, S], BF16)
        wq_sb = kq_pool.tile([P, KT_H, HD], BF16)
        wv_sb = wv_pool.tile([P, KT_H, HD], BF16)
        # one batched DMA per tensor (per seq-chunk for x): issuing from the
        # SP queue costs ~565ns per dma_start, so 24 small DMAs would pace
        # the whole projection head
        wk_r = wk_d.rearrange("(t p) c -> p t c", p=P)
        xT_r = xT_d.rearrange("(t p) s -> p t s", p=P)
        # chunk 0 and wk arrive in k-tile pieces, wq between them: the K
        # projection needs only wk + x, so the first matmuls start ~3µs in
        nc.sync.dma_start(out=wk_sb[:, 0:4, :], in_=wk_r[:, 0:4, :])
        nc.sync.dma_start(out=xT_sb[:, 0:2, 0:512], in_=xT_r[:, 0:2, 0:512])
        nc.sync.dma_start(out=xT_sb[:, 2:4, 0:512], in_=xT_r[:, 2:4, 0:512])
        nc.sync.dma_start(out=wk_sb[:, 4:8, :], in_=wk_r[:, 4:8, :])
        nc.sync.dma_start(out=wq_sb[:], in_=wq_d.rearrange("(t p) c -> p t c", p=P))
        nc.sync.dma_start(out=xT_sb[:, 4:8, 0:512], in_=xT_r[:, 4:8, 0:512])
        for cc in range(1, 4):
            nc.sync.dma_start(out=xT_sb[:, :, cc * 512:(cc + 1) * 512],
                              in_=xT_r[:, :, cc * 512:(cc + 1) * 512])
        nc.sync.dma_start(out=wv_sb[:], in_=wv_d.rearrange("(t p) c -> p t c", p=P))

        # all projection psum traffic lives in two persistent banks (tags
        # ps_k / ps_q, alternating for double-buffering) so the attention
        # pool can hold its six banks for the whole kernel with no
        # pool-boundary barrier between projections and attention
        def qk_proj(ps_pool, w_sb, b_sb, dst, p):
            for c4 in range(4):
                ps_qk = ps_pool.tile([P, 512], F32, bufs=1, name="ps_qk",
                                     tag="ps_k" if c4 % 2 == 0 else "ps_q")
                for kt in range(KT_H):
                    mm(ps_qk[:],
                       w_sb[:, kt, p * P:(p + 1) * P],
                       xT_sb[:, kt, c4 * 512:(c4 + 1) * 512],
                       start=(kt == 0), stop=(not with_bias and kt == KT_H - 1))
                if with_bias:
                    mm(ps_qk[:],
                       b_sb[:, p * P:(p + 1) * P],
                       ones_sb[:, 0:512],
                       start=False, stop=True)
                nc.vector.tensor_copy(dst[:, p, c4 * 512:(c4 + 1) * 512], ps_qk[:])

        def oproj_quarter(o_ps, opool, q, pre=None, tmp_f=None):
            # psum->sbuf copies split across ACT and DVE; each half-tile
            # DMAs out as soon as it is copied.  For the final quarter the
            # pair-1 odd ctx half never went through the bounce DMA: it is
            # read from tmp_f (partitions 0-63) as an extra K=64 step, and
            # the first m-tile's pair-0 matmuls were pre-emitted (`pre`)
            # ahead of the norm chain so the PE keeps streaming through it.
            final = (q == NCH - 1)
            for m in range(4 * q, 4 * q + 4):
                o_sb = opool.tile([P, H], F32, tag="o_sb", bufs=3)
                for n2 in range(2):
                    ns = slice(n2 * 512, (n2 + 1) * 512)
                    if pre is not None and m == 4 * q:
                        ps_o = pre[n2]
                    else:
                        ps_o = o_ps.tile([P, 512], F32, tag="ps_o", bufs=2)
                        mm(ps_o[:], ctxT_sb[:, 0, m * P:(m + 1) * P],
                           wo_sb[:, 0, ns], start=True, stop=False)
                    if final:
                        # read the odd ctx half straight from tmp_f — the
                        # final quarter must not wait on the bounce DMA
                        mm(ps_o[:], ctxT_sb[0:64, 1, m * P:(m + 1) * P],
                           wo_sb[0:64, 1, ns], start=False, stop=False)
                        mc = (m - 4 * q) * P
                        mm(ps_o[:], tmp_f[:, mc:mc + P],
                           wo_odd[:, ns], start=False, stop=True)
                    else:
                        mm(ps_o[:], ctxT_sb[:, 1, m * P:(m + 1) * P],
                           wo_sb[:, 1, ns], start=False, stop=True)
                    if n2 == 0:
                        nc.scalar.copy(o_sb[:, 0:512], ps_o[:])
                    else:
                        nc.vector.tensor_copy(o_sb[:, 512:1024], ps_o[:])
                    nc.sync.dma_start(
                        out=o_d[m * P:(m + 1) * P, ns],
                        in_=o_sb[:, ns])

        def v_proj(ms, pool):
            for m in ms:
                ps_v = pool.tile([P, HD], F32, bufs=1, name="ps_v",
                                 tag="ps_k" if m % 2 == 0 else "ps_q")
                for kt in range(KT_H):
                    mm(ps_v[:],
                       xT_sb[:, kt, m * P:(m + 1) * P],
                       wv_sb[:, kt, :],
                       start=(kt == 0), stop=(not with_bias and kt == KT_H - 1))
                if with_bias:
                    mm(ps_v[:],
                       ones_sb[:, 0:P],
                       bv_sb[:],
                       start=False, stop=True)
                nc.vector.tensor_copy(v_sb[:, m, :, 0:HEAD_DIM], ps_v[:])

        # ---------- pools: attention psum first (banks 0-5, alive for the
        # whole kernel), projections in the remaining two banks ----------
        attn_stack = contextlib.ExitStack()
        a_ps = attn_stack.enter_context(
            tc.tile_pool(name="attn_psum", bufs=1, space="PSUM"))
        ptp = attn_stack.enter_context(
            tc.tile_pool(name="pt_pool", bufs=8 if not (masked or with_bias) else 2))
        npool = attn_stack.enter_context(tc.tile_pool(name="norm_pool", bufs=2))
        psA = tc.alloc_tile_pool(name="proj_psum", bufs=1, space="PSUM")

        # pair-0 K/Q projections, seq-chunk-outer so each inbound x
        # column-chunk is consumed as soon as it lands; score tiles for
        # seq-chunk 0 can then start ~3MB into the inbound DMA
        for cc in range(4):
            ps_k = psA.tile([P, 512], F32, tag="ps_k", bufs=1, name="ps_k")
            ps_q = psA.tile([P, 512], F32, tag="ps_q", bufs=1, name="ps_q")
            # chunk 0 runs K fully before Q (K needs only wk + x; wq is
            # still inbound); later chunks interleave for psum parallelism
            pairs = (((ps_k, wk_sb),), ((ps_q, wq_sb),)) if cc == 0 \
                else (((ps_k, wk_sb), (ps_q, wq_sb)),)
            for group in pairs:
                for kt in range(KT_H):
                    for ps, w_sb in group:
                        mm(ps[:],
                           w_sb[:, kt, 0:P],
                           xT_sb[:, kt, cc * 512:(cc + 1) * 512],
                           start=(kt == 0),
                           stop=(not with_bias and kt == KT_H - 1))
            if with_bias:
                for ps, b_sb in ((ps_k, bk_sb), (ps_q, bq_sb)):
                    mm(ps[:],
                       b_sb[:, 0:P],
                       ones_sb[:, 0:512],
                       start=False, stop=True)
            nc.vector.tensor_copy(kT_sb[:, 0, cc * 512:(cc + 1) * 512], ps_k[:])
            nc.vector.tensor_copy(qT_sb[:, 0, cc * 512:(cc + 1) * 512], ps_q[:])

        v_proj(range(KT_S), psA)

        # pair-0 attention; V projection + pair-1 projections fill the PE
        # shadow under the ACT-bound softmax
        for c in range(NCH):
            _attn_one_chunk(tc, nc, a_ps, ptp, npool, 0, c, masked,
                            amask_sb if masked else None,
                            kT_sb, qT_sb, v_sb, ctxT_sb, ones64)
        qk_proj(psA, wk_sb, bk_sb if with_bias else None, kT_sb, 1)
        qk_proj(psA, wq_sb, bq_sb if with_bias else None, qT_sb, 1)
        wv_stack.close()

        # wo arrives during pair-0 attention; needed only in the final phase
        nc.sync.dma_start(out=wo_sb[:], in_=wo_d.rearrange("(t p) c -> p t c", p=P))
        # pair-1 odd-head rows of wo, lowered to partitions 0-63: lets the
        # final chunk's output projection read the un-bounced odd ctx (which
        # lives at partitions 0-63) as a separate K=64 accumulation step
        wo_odd = const.tile([64, H], F32R)
        nc.sync.dma_start(out=wo_odd[:], in_=wo_sb[64:128, 1, :])
        kq_pool.release()
        psA.release()

        # pair-1 attention with the output projection interleaved per chunk
        o_ps = attn_stack.enter_context(tc.tile_pool(name="o_psum", bufs=1, space="PSUM"))
        opool = attn_stack.enter_context(tc.tile_pool(name="o_pool", bufs=1))
        for c in range(NCH):
            ctxu1, recip1 = _attn_one_chunk(
                tc, nc, a_ps, ptp, npool, 1, c, masked,
                amask_sb if masked else None,
                kT_sb, qT_sb, v_sb, ctxT_sb, ones64)
            # slot the first m-tile's pair-0 matmuls in front of the norm
            # chain (bc/mul/bounce) the PE would otherwise wait on
            pre = []
            for n2 in range(2):
                ps_p = o_ps.tile([P, 512], F32, tag="ps_o", bufs=2,
                                 name="ps_p")
                mm(ps_p[:], ctxT_sb[:, 0, 4 * c * P:(4 * c + 1) * P],
                   wo_sb[:, 0, n2 * 512:(n2 + 1) * 512],
                   start=True, stop=False)
                pre.append(ps_p)
            tmp_f = _norm_finish(nc, a_ps, npool, 1, c, ctxu1, recip1,
                                 ctxT_sb, ones64)
            oproj_quarter(o_ps, opool, c, pre, tmp_f)
        attn_stack.close()


def _attn_one_chunk(tc, nc, psum, ptp, npool, p, c, masked, amask_sb,
                    kT_sb, qT_sb, v_sb, ctxT_sb, ones64):
    mm = nc.tensor.matmul
    ctx_e = psum.tile([HEAD_DIM + 1, CHUNK], F32, tag="ctx_e", bufs=1)
    ctx_o = psum.tile([HEAD_DIM + 1, CHUNK], F32, tag="ctx_o", bufs=1)
    sched = EXP_SCHED[p]
    for kt in range(KT_S):
        # separate per-head score tiles: each half releases to the next
        # score matmul as soon as ITS exp drains, halving pipeline stalls
        s_half = [psum.tile([P, CHUNK], F32, tag=f"s_{hl}", bufs=2, name=f"s_{hl}")
                  for hl in range(2)]
        for hl in range(2):
            mm(s_half[hl][:],
               kT_sb[hl * 64:(hl + 1) * 64, p, kt * P:(kt + 1) * P],
               qT_sb[hl * 64:(hl + 1) * 64, p, c * CHUNK:(c + 1) * CHUNK],
               start=True, stop=True)
        pt = ptp.tile([P, 2 * CHUNK], BF16, tag="pt")
        for hl in range(2):
            dst = pt[:, hl * CHUNK:(hl + 1) * CHUNK]
            if masked:
                nc.scalar.activation(dst, s_half[hl][:], EXP,
                                     bias=amask_sb[:, kt:kt + 1])
            elif sched[(kt, hl)] == 'A':
                nc.scalar.activation(dst, s_half[hl][:], EXP)
            else:
                # softmax exp off the ACT engine: bitcast-int16 Schraudolph
                # (~1.5% rms ripple; softmax's ratio cancels the mean shift)
                eng = nc.vector if sched[(kt, hl)] == 'V' else nc.gpsimd
                eng.tensor_scalar(dst.bitcast(I16), s_half[hl][:], SCH_A, SCH_B,
                                  mybir.AluOpType.mult, mybir.AluOpType.add)
        for hl in range(2):
            ctx = ctx_e if hl == 0 else ctx_o
            mm(ctx[:],
               v_sb[:, kt, 2 * p + hl, :],
               pt[:, hl * CHUNK:(hl + 1) * CHUNK],
               start=(kt == 0), stop=(kt == KT_S - 1))
    # chunk-end psum drain off the DVE (a DVE burst here head-of-line-blocks
    # the next chunk's exp tiles); pair-1 splits it ACT/Pool to halve either
    # queue's burst
    ctxu = npool.tile([HEAD_DIM + 1, 2, CHUNK], F32, tag="ctxu", bufs=2)
    nc.scalar.copy(ctxu[:, 0, :], ctx_e[:])
    if p == 0:
        nc.scalar.copy(ctxu[:, 1, :], ctx_o[:])
    else:
        nc.vector.tensor_copy(ctxu[:, 1, :], ctx_o[:])
    recip_sb = npool.tile([HEAD_DIM + 1, 2, CHUNK], F32R, tag="recip", bufs=2)
    with nc.allow_low_precision(reason="softmax denominators are O(1e3); fp32r's 11-bit mantissa is plenty"):
        nc.vector.reciprocal(recip_sb[64:65, :, :], ctxu[64:65, :, :])
    if p == 1:
        # caller emits latency-tolerant oproj matmuls before the bc chain
        return ctxu, recip_sb
    _norm_finish(nc, psum, npool, p, c, ctxu, recip_sb, ctxT_sb, ones64)
    return None


def _norm_finish(nc, psum, npool, p, c, ctxu, recip_sb, ctxT_sb, ones64):
    mm = nc.tensor.matmul
    bc_e = psum.tile([HEAD_DIM, CHUNK], F32, tag="ctx_e", bufs=1)
    bc_o = psum.tile([HEAD_DIM, CHUNK], F32, tag="ctx_o", bufs=1)
    for hl in range(2):
        mm(bc_e if hl == 0 else bc_o,
           ones64[64:65, :],
           recip_sb[64:65, hl, :],
           start=True, stop=True)
    if p == 1 and c == NCH - 1:
        # final chunk: the even half writes in place (partition-aligned);
        # the odd half skips the ~2.5µs bounce-DMA chain — the output
        # projection reads tmp_f directly as an extra K=64 step
        nc.vector.tensor_mul(ctxT_sb[0:64, p, c * CHUNK:(c + 1) * CHUNK],
                             ctxu[0:64, 0, :], bc_e[:])
        tmp_f = npool.tile([HEAD_DIM, CHUNK], F32R, tag="tmp_o", bufs=2)
        nc.vector.tensor_mul(tmp_f[:], ctxu[0:64, 1, :], bc_o[:])
        return tmp_f
    # odd-half mul first: its bounce DMA is on the next oproj quarter's
    # critical path, the even half's in-place write is not
    tmp_o = npool.tile([HEAD_DIM, CHUNK], F32R, tag="tmp_o", bufs=2)
    nc.vector.tensor_mul(tmp_o[:], ctxu[0:64, 1, :], bc_o[:])
    # partition-shifting bounce must be a DMA: engine lanes are
    # partition-locked, and GPSIMD cannot access PSUM on TRN2 anyway
    nc.sync.dma_start(out=ctxT_sb[64:128, p, c * CHUNK:(c + 1) * CHUNK],
                      in_=tmp_o[:])
    nc.vector.tensor_mul(ctxT_sb[0:64, p, c * CHUNK:(c + 1) * CHUNK],
                         ctxu[0:64, 0, :], bc_e[:])
    return tmp_o


def build_program(masked=False, with_bias=False):
    key = (masked, with_bias)
    if key in _PROGRAM_CACHE:
        return _PROGRAM_CACHE[key]
    nc = bacc.Bacc("TRN2", target_bir_lowering=False, debug=False,
                   enable_asserts=False)
    xT = nc.dram_tensor("xT", [H, S], BF16, kind="ExternalInput").ap()
    wq = nc.dram_tensor("wq", [H, HD], BF16, kind="ExternalInput").ap()
    wk = nc.dram_tensor("wk", [H, HD], BF16, kind="ExternalInput").ap()
    wv = nc.dram_tensor("wv", [H, HD], BF16, kind="ExternalInput").ap()
    wo = nc.dram_tensor("wo", [HD, H], F32R, kind="ExternalInput").ap()
    bq = nc.dram_tensor("bq", [1, HD], F32R, kind="ExternalInput").ap()
    bk = nc.dram_tensor("bk", [1, HD], F32R, kind="ExternalInput").ap()
    bv = nc.dram_tensor("bv", [1, HD], F32R, kind="ExternalInput").ap()
    am = nc.dram_tensor("am", [P, KT_S], F32, kind="ExternalInput").ap()
    o = nc.dram_tensor("o_part", [S, H], F32, kind="ExternalOutput").ap()
    with tile.TileContext(nc) as tc:
        _emit(tc, nc, (xT, wq, wk, wv, wo, bq, bk, bv, am, o), masked, with_bias)
    nc.compile()
    _PROGRAM_CACHE[key] = nc
    return nc


def _round_fp32r(a):
    """Round fp32 to the PE's fp32r format (11 mantissa bits, RNE)."""
    u = np.ascontiguousarray(a, np.float32).view(np.uint32)
    r = (u + np.uint32(0x7FF) + ((u >> np.uint32(12)) & np.uint32(1))) \
        & np.uint32(0xFFFFF000)
    return r.view(np.float32)


def make_in_maps(hidden_states, attention_mask, Wq, bq, Wk, bk, Wv, bv, Wo, bo):
    """Per-core input dicts. Core c: batch c//4, head-group c%4.

    Wq/bq are pre-scaled by 1/8 (= 1/sqrt(HEAD_DIM), exact in fp32) so the
    kernel's raw scores are already scaled. Tensors feeding float32r
    matmuls are pre-rounded to fp32r on the host (the device DMAs them
    into float32r tiles verbatim).
    """
    import ml_dtypes
    bf16 = ml_dtypes.bfloat16
    hidden_states = np.asarray(hidden_states, np.float32)
    attention_mask = np.asarray(attention_mask, np.float32)
    xTs = [np.ascontiguousarray(hidden_states[b].T).astype(bf16) for b in range(B)]
    ams = []
    for b in range(B):
        amask = ((1.0 - attention_mask[b]) * -10000.0).astype(np.float32)
        ams.append(np.ascontiguousarray(amask.reshape(KT_S, P).T))
    in_maps = []
    for c in range(N_CORES):
        b, g = divmod(c, GROUPS)
        hs = slice(g * HD, (g + 1) * HD)
        in_maps.append({
            "xT": xTs[b],
            "wq": (np.asarray(Wq, np.float32)[hs, :].T * np.float32(0.125)).astype(bf16),
            "wk": np.asarray(Wk, np.float32)[hs, :].T.astype(bf16),
            "wv": np.asarray(Wv, np.float32)[hs, :].T.astype(bf16),
            "wo": _round_fp32r(np.asarray(Wo, np.float32)[:, hs].T),
            "bq": _round_fp32r(np.asarray(bq, np.float32)[hs].reshape(1, HD) * np.float32(0.125)),
            "bk": _round_fp32r(np.asarray(bk, np.float32)[hs].reshape(1, HD)),
            "bv": _round_fp32r(np.asarray(bv, np.float32)[hs].reshape(1, HD)),
            "am": ams[b],
        })
    return in_maps


def kernel(hidden_states, attention_mask, Wq, bq, Wk, bk, Wv, bv, Wo, bo):
    masked = not bool(np.all(np.asarray(attention_mask) == 1.0))
    with_bias = not (np.all(np.asarray(bq) == 0) and np.all(np.asarray(bk) == 0)
                     and np.all(np.asarray(bv) == 0))
    nc = build_program(masked, with_bias)
    in_maps = make_in_maps(hidden_states, attention_mask,
                           Wq, bq, Wk, bk, Wv, bv, Wo, bo)
    res = run_bass_kernel_spmd(nc, in_maps, core_ids=list(range(N_CORES)))
    out = np.zeros((B, S, H), np.float32)
    for c in range(N_CORES):
        b = c // GROUPS
        out[b] += res.results[c]["o_part"]
    out += np.asarray(bo, np.float32)
    return out



# revision 54
# speedup vs baseline: 1.0001x; 1.0001x over previous
"""Trainium2 Bass kernel: 16-head attention block (B=2, S=2048, H=1024).

Sharding: 8 cores = 2-way data parallel (batch) x 4-way tensor parallel
(head groups of 4 heads / 256 dims). Each core computes, for its batch
and head group:
    Q^T, K^T (= W @ x^T, [dims, seq] layout; Wq/bq pre-scaled by 1/8 on
    host so no score scaling is needed on device), V ([seq, dims]),
    S^T = K Q^T per head (key positions on partitions),
    P^T = exp(S^T + mask),
    ctx'^T = [V | 1]^T P^T    (ones column folded in -> row 64 = softmax
                               denominator),
    ctx^T normalized, then partial output O_g = ctx^T.T @ Wo[:,hs]^T.
Host sums the 4 partial outputs per batch and adds bo.

Precision: x and the QKV weights ship as bf16 (halves the inbound DMA);
scores run as float32r; the probabilities P and V are bf16 for the ctx
matmul; the output projection is float32r.  3/8 of the softmax exp
half-tiles run as an int16 Schraudolph (bitcast) on the otherwise-
saturated DVE instead of the ACT engine.  Measured end-to-end rel err
~1.0e-2 against the fp32 reference (gate: 2e-2).

Overlap: emission order interleaves pair-1 projections under pair-0's
softmax shadow and the output projection under pair-1's; per-head score
psum tiles + a 6-deep pt pool keep the PE's score->exp->ctx pipeline
from stalling on any single engine's queue.
"""

import contextlib

import numpy as np

import concourse.bass as bass
import concourse.mybir as mybir
import concourse.tile as tile
from concourse import bacc
from concourse.bass_utils import run_bass_kernel_spmd

B, S, H = 2, 2048, 1024
NUM_HEADS, HEAD_DIM = 16, 64
N_CORES = 8
GROUPS = 4                  # head-parallel groups per batch
HD = H // GROUPS            # 256 head-dims per core (4 heads)
P = 128
KT_H = H // P               # 8 k-tiles over hidden dim
KT_S = S // P               # 16 k-tiles over sequence (key positions)
NCH = 4                     # q chunks
CHUNK = S // NCH            # 512
F32 = mybir.dt.float32
F32R = mybir.dt.float32r
BF16 = mybir.dt.bfloat16
I16 = mybir.dt.int16
EXP = mybir.ActivationFunctionType.Exp

# exp(s) ~= bitcast_bf16(int16(s * 2^7/ln2 + (127*2^7 - c))).  c tuned for
# min-rms relative ripple of the linear-mantissa approximation (~1.5% rms).
# The DVE/Pool f32->i16 convert is round-to-nearest (verified on device);
# softmax's ratio cancels the uniform component, leaving only the ripple.
SCH_A = float(128.0 / np.log(2.0))
SCH_B = float(127.0 * 128.0 - 0.043677448 * 128.0)
# Per-(kt, head-half) exp engine: 'A' = ACT (exact exp), 'V' = DVE
# (int16 Schraudolph).  The Pool engine cannot touch PSUM on TRN2, so it
# only carries SBUF->SBUF traffic (the ctxT bounce copies).  ACT keeps
# 20/32 half-tiles; the DVE share is sized so neither queue bursts past
# the PE's per-kt pace.
def _exp_sched(p):
    sched = {}
    dve_o = (1, 3, 5, 7, 9, 11, 13, 15)
    dve_e = (2, 6, 10, 14)
    for kt in range(KT_S):
        sched[(kt, 0)] = 'V' if kt in dve_e else 'A'
        sched[(kt, 1)] = 'V' if kt in dve_o else 'A'
    return sched


EXP_SCHED = (_exp_sched(0), _exp_sched(1))

_PROGRAM_CACHE = {}


def _emit(tc, nc, dram, masked, with_bias):
    mm = nc.tensor.matmul
    xT_d, wq_d, wk_d, wv_d, wo_d, bq_d, bk_d, bv_d, am_d, o_d = dram

    stack = contextlib.ExitStack()
    with stack:
        const = stack.enter_context(tc.tile_pool(name="const", bufs=1))
        big = stack.enter_context(tc.tile_pool(name="big", bufs=1))

        onesf = const.tile([P, 64], F32)
        nc.any.memset(onesf[:], 1.0)
        ones64 = const.tile([P, 64], F32R)   # lane-64 row used as K=1 lhsT
        nc.vector.tensor_copy(ones64[:], onesf[:])
        onesb = const.tile([P, 64], BF16)
        nc.vector.tensor_copy(onesb[:], onesf[:])
        # warm the ACT exp table before it is first needed
        trash = const.tile([1, 16], F32)
        nc.scalar.activation(trash[:], onesf[0:1, 0:16], EXP)
        if masked:
            amask_sb = const.tile([P, KT_S], F32)
            nc.sync.dma_start(out=amask_sb[:], in_=am_d[:])
        if with_bias:
            ones_sb = const.tile([1, 512], F32R)
            for i in range(8):
                nc.vector.tensor_copy(ones_sb[0:1, i * 64:(i + 1) * 64],
                                      onesf[0:1, :])
            bq_sb = const.tile([1, HD], F32R)
            nc.sync.dma_start(out=bq_sb[:], in_=bq_d[:])
            bk_sb = const.tile([1, HD], F32R)
            nc.sync.dma_start(out=bk_sb[:], in_=bk_d[:])
            bv_sb = const.tile([1, HD], F32R)
            nc.sync.dma_start(out=bv_sb[:], in_=bv_d[:])
        wo_sb = const.tile([P, HD // P, H], F32R)

        # persistent activations
        qT_sb = big.tile([P, 2, S], F32R)    # [dim-in-pair, pair, seq]
        kT_sb = big.tile([P, 2, S], F32R)
        v_sb = big.tile([P, KT_S, GROUPS, HEAD_DIM + 1], BF16)  # [seq, kt, head, d+1]
        ctxT_sb = big.tile([P, 2, S], F32R)

        # ones column of V' (the rowsum trick)
        nc.vector.tensor_copy(v_sb[:, :, :, HEAD_DIM:HEAD_DIM + 1], onesb[:, 0:KT_S * GROUPS])

        # ---------- input DMAs, interleaved k-tile-wise so the pair-0 K/Q
        # projection for seq-chunk 0 can start ~0.5MB into the inbound
        # stream instead of after all of wk+wq (4µs vs 14µs) ----------
        kq_pool = tc.alloc_tile_pool(name="kq_pool", bufs=1, side="right")
        wv_stack = contextlib.ExitStack()
        wv_pool = wv_stack.enter_context(tc.tile_pool(name="wv_pool", bufs=1, side="right"))
        wk_sb = kq_pool.tile([P, KT_H, HD], BF16)
        xT_sb = kq_pool.tile([P, KT_H, S], BF16)
        wq_sb = kq_pool.tile([P, KT_H, HD], BF16)
        wv_sb = wv_pool.tile([P, KT_H, HD], BF16)
        # one batched DMA per tensor (per seq-chunk for x): issuing from the
        # SP queue costs ~565ns per dma_start, so 24 small DMAs would pace
        # the whole projection head
        wk_r = wk_d.rearrange("(t p) c -> p t c", p=P)
        xT_r = xT_d.rearrange("(t p) s -> p t s", p=P)
        # chunk 0 and wk arrive in k-tile pieces, wq between them: the K
        # projection needs only wk + x, so the first matmuls start ~3µs in
        nc.sync.dma_start(out=wk_sb[:, 0:4, :], in_=wk_r[:, 0:4, :])
        nc.sync.dma_start(out=xT_sb[:, 0:2, 0:512], in_=xT_r[:, 0:2, 0:512])
        nc.sync.dma_start(out=xT_sb[:, 2:4, 0:512], in_=xT_r[:, 2:4, 0:512])
        nc.sync.dma_start(out=wk_sb[:, 4:8, :], in_=wk_r[:, 4:8, :])
        nc.sync.dma_start(out=wq_sb[:], in_=wq_d.rearrange("(t p) c -> p t c", p=P))
        nc.sync.dma_start(out=xT_sb[:, 4:8, 0:512], in_=xT_r[:, 4:8, 0:512])
        for cc in range(1, 4):
            nc.sync.dma_start(out=xT_sb[:, :, cc * 512:(cc + 1) * 512],
                              in_=xT_r[:, :, cc * 512:(cc + 1) * 512])
        nc.sync.dma_start(out=wv_sb[:], in_=wv_d.rearrange("(t p) c -> p t c", p=P))

        # all projection psum traffic lives in two persistent banks (tags
        # ps_k / ps_q, alternating for double-buffering) so the attention
        # pool can hold its six banks for the whole kernel with no
        # pool-boundary barrier between projections and attention
        def qk_proj(ps_pool, w_sb, b_sb, dst, p):
            for c4 in range(4):
                ps_qk = ps_pool.tile([P, 512], F32, bufs=1, name="ps_qk",
                                     tag="ps_k" if c4 % 2 == 0 else "ps_q")
                for kt in range(KT_H):
                    mm(ps_qk[:],
                       w_sb[:, kt, p * P:(p + 1) * P],
                       xT_sb[:, kt, c4 * 512:(c4 + 1) * 512],
                       start=(kt == 0), stop=(not with_bias and kt == KT_H - 1))
                if with_bias:
                    mm(ps_qk[:],
                       b_sb[:, p * P:(p + 1) * P],
                       ones_sb[:, 0:512],
                       start=False, stop=True)
                nc.vector.tensor_copy(dst[:, p, c4 * 512:(c4 + 1) * 512], ps_qk[:])

        def oproj_quarter(o_ps, opool, q, pre=None, tmp_f=None):
            # psum->sbuf copies split across ACT and DVE; each half-tile
            # DMAs out as soon as it is copied.  For the final quarter the
            # pair-1 odd ctx half never went through the bounce DMA: it is
            # read from tmp_f (partitions 0-63) as an extra K=64 step, and
            # the first m-tile's pair-0 matmuls were pre-emitted (`pre`)
            # ahead of the norm chain so the PE keeps streaming through it.
            final = (q == NCH - 1)
            for m in range(4 * q, 4 * q + 4):
                o_sb = opool.tile([P, H], F32, tag="o_sb", bufs=3)
                for n2 in range(2):
                    ns = slice(n2 * 512, (n2 + 1) * 512)
                    if pre is not None and m == 4 * q:
                        ps_o = pre[n2]
                    else:
                        ps_o = o_ps.tile([P, 512], F32, tag="ps_o", bufs=2)
                        mm(ps_o[:], ctxT_sb[:, 0, m * P:(m + 1) * P],
                           wo_sb[:, 0, ns], start=True, stop=False)
                    if final:
                        # read the odd ctx half straight from tmp_f — the
                        # final quarter must not wait on the bounce DMA
                        mm(ps_o[:], ctxT_sb[0:64, 1, m * P:(m + 1) * P],
                           wo_sb[0:64, 1, ns], start=False, stop=False)
                        mc = (m - 4 * q) * P
                        mm(ps_o[:], tmp_f[:, mc:mc + P],
                           wo_odd[:, ns], start=False, stop=True)
                    else:
                        mm(ps_o[:], ctxT_sb[:, 1, m * P:(m + 1) * P],
                           wo_sb[:, 1, ns], start=False, stop=True)
                    if n2 == 0:
                        nc.scalar.copy(o_sb[:, 0:512], ps_o[:])
                    else:
                        nc.vector.tensor_copy(o_sb[:, 512:1024], ps_o[:])
                    nc.sync.dma_start(
                        out=o_d[m * P:(m + 1) * P, ns],
                        in_=o_sb[:, ns])

        def v_proj(ms, pool):
            for m in ms:
                ps_v = pool.tile([P, HD], F32, bufs=1, name="ps_v",
                                 tag="ps_k" if m % 2 == 0 else "ps_q")
                for kt in range(KT_H):
                    mm(ps_v[:],
                       xT_sb[:, kt, m * P:(m + 1) * P],
                       wv_sb[:, kt, :],
                       start=(kt == 0), stop=(not with_bias and kt == KT_H - 1))
                if with_bias:
                    mm(ps_v[:],
                       ones_sb[:, 0:P],
                       bv_sb[:],
                       start=False, stop=True)
                nc.vector.tensor_copy(v_sb[:, m, :, 0:HEAD_DIM], ps_v[:])

        # ---------- pools: attention psum first (banks 0-5, alive for the
        # whole kernel), projections in the remaining two banks ----------
        attn_stack = contextlib.ExitStack()
        a_ps = attn_stack.enter_context(
            tc.tile_pool(name="attn_psum", bufs=1, space="PSUM"))
        ptp = attn_stack.enter_context(
            tc.tile_pool(name="pt_pool", bufs=6 if not (masked or with_bias) else 2))
        npool = attn_stack.enter_context(tc.tile_pool(name="norm_pool", bufs=2))
        psA = tc.alloc_tile_pool(name="proj_psum", bufs=1, space="PSUM")

        # pair-0 K/Q projections, seq-chunk-outer so each inbound x
        # column-chunk is consumed as soon as it lands; score tiles for
        # seq-chunk 0 can then start ~3MB into the inbound DMA
        for cc in range(4):
            ps_k = psA.tile([P, 512], F32, tag="ps_k", bufs=1, name="ps_k")
            ps_q = psA.tile([P, 512], F32, tag="ps_q", bufs=1, name="ps_q")
            # chunk 0 runs K fully before Q (K needs only wk + x; wq is
            # still inbound); later chunks interleave for psum parallelism
            pairs = (((ps_k, wk_sb),), ((ps_q, wq_sb),)) if cc == 0 \
                else (((ps_k, wk_sb), (ps_q, wq_sb)),)
            for group in pairs:
                for kt in range(KT_H):
                    for ps, w_sb in group:
                        mm(ps[:],
                           w_sb[:, kt, 0:P],
                           xT_sb[:, kt, cc * 512:(cc + 1) * 512],
                           start=(kt == 0),
                           stop=(not with_bias and kt == KT_H - 1))
            if with_bias:
                for ps, b_sb in ((ps_k, bk_sb), (ps_q, bq_sb)):
                    mm(ps[:],
                       b_sb[:, 0:P],
                       ones_sb[:, 0:512],
                       start=False, stop=True)
            nc.vector.tensor_copy(kT_sb[:, 0, cc * 512:(cc + 1) * 512], ps_k[:])
            nc.vector.tensor_copy(qT_sb[:, 0, cc * 512:(cc + 1) * 512], ps_q[:])

        v_proj(range(KT_S), psA)

        # pair-0 attention; V projection + pair-1 projections fill the PE
        # shadow under the ACT-bound softmax
        for c in range(NCH):
            _attn_one_chunk(tc, nc, a_ps, ptp, npool, 0, c, masked,
                            amask_sb if masked else None,
                            kT_sb, qT_sb, v_sb, ctxT_sb, ones64)
        qk_proj(psA, wk_sb, bk_sb if with_bias else None, kT_sb, 1)
        qk_proj(psA, wq_sb, bq_sb if with_bias else None, qT_sb, 1)
        wv_stack.close()

        # wo arrives during pair-0 attention; needed only in the final phase
        nc.sync.dma_start(out=wo_sb[:], in_=wo_d.rearrange("(t p) c -> p t c", p=P))
        # pair-1 odd-head rows of wo, lowered to partitions 0-63: lets the
        # final chunk's output projection read the un-bounced odd ctx (which
        # lives at partitions 0-63) as a separate K=64 accumulation step
        wo_odd = const.tile([64, H], F32R)
        nc.sync.dma_start(out=wo_odd[:], in_=wo_sb[64:128, 1, :])
        kq_pool.release()
        psA.release()

        # pair-1 attention with the output projection interleaved per chunk
        o_ps = attn_stack.enter_context(tc.tile_pool(name="o_psum", bufs=1, space="PSUM"))
        opool = attn_stack.enter_context(tc.tile_pool(name="o_pool", bufs=1))
        for c in range(NCH):
            ctxu1, recip1 = _attn_one_chunk(
                tc, nc, a_ps, ptp, npool, 1, c, masked,
                amask_sb if masked else None,
                kT_sb, qT_sb, v_sb, ctxT_sb, ones64)
            # slot the first m-tile's pair-0 matmuls in front of the norm
            # chain (bc/mul/bounce) the PE would otherwise wait on
            pre = []
            for n2 in range(2):
                ps_p = o_ps.tile([P, 512], F32, tag="ps_o", bufs=2,
                                 name="ps_p")
                mm(ps_p[:], ctxT_sb[:, 0, 4 * c * P:(4 * c + 1) * P],
                   wo_sb[:, 0, n2 * 512:(n2 + 1) * 512],
                   start=True, stop=False)
                pre.append(ps_p)
            tmp_f = _norm_finish(nc, a_ps, npool, 1, c, ctxu1, recip1,
                                 ctxT_sb, ones64)
            oproj_quarter(o_ps, opool, c, pre, tmp_f)
        attn_stack.close()


def _attn_one_chunk(tc, nc, psum, ptp, npool, p, c, masked, amask_sb,
                    kT_sb, qT_sb, v_sb, ctxT_sb, ones64):
    mm = nc.tensor.matmul
    ctx_e = psum.tile([HEAD_DIM + 1, CHUNK], F32, tag="ctx_e", bufs=1)
    ctx_o = psum.tile([HEAD_DIM + 1, CHUNK], F32, tag="ctx_o", bufs=1)
    sched = EXP_SCHED[p]
    for kt in range(KT_S):
        # separate per-head score tiles: each half releases to the next
        # score matmul as soon as ITS exp drains, halving pipeline stalls
        s_half = [psum.tile([P, CHUNK], F32, tag=f"s_{hl}", bufs=2, name=f"s_{hl}")
                  for hl in range(2)]
        for hl in range(2):
            mm(s_half[hl][:],
               kT_sb[hl * 64:(hl + 1) * 64, p, kt * P:(kt + 1) * P],
               qT_sb[hl * 64:(hl + 1) * 64, p, c * CHUNK:(c + 1) * CHUNK],
               start=True, stop=True)
        pt = ptp.tile([P, 2 * CHUNK], BF16, tag="pt")
        for hl in range(2):
            dst = pt[:, hl * CHUNK:(hl + 1) * CHUNK]
            if masked:
                nc.scalar.activation(dst, s_half[hl][:], EXP,
                                     bias=amask_sb[:, kt:kt + 1])
            elif sched[(kt, hl)] == 'A':
                nc.scalar.activation(dst, s_half[hl][:], EXP)
            else:
                # softmax exp off the ACT engine: bitcast-int16 Schraudolph
                # (~1.5% rms ripple; softmax's ratio cancels the mean shift)
                eng = nc.vector if sched[(kt, hl)] == 'V' else nc.gpsimd
                eng.tensor_scalar(dst.bitcast(I16), s_half[hl][:], SCH_A, SCH_B,
                                  mybir.AluOpType.mult, mybir.AluOpType.add)
        for hl in range(2):
            ctx = ctx_e if hl == 0 else ctx_o
            mm(ctx[:],
               v_sb[:, kt, 2 * p + hl, :],
               pt[:, hl * CHUNK:(hl + 1) * CHUNK],
               start=(kt == 0), stop=(kt == KT_S - 1))
    # chunk-end psum drain off the DVE (a DVE burst here head-of-line-blocks
    # the next chunk's exp tiles); pair-1 splits it ACT/Pool to halve either
    # queue's burst
    ctxu = npool.tile([HEAD_DIM + 1, 2, CHUNK], F32, tag="ctxu", bufs=2)
    nc.scalar.copy(ctxu[:, 0, :], ctx_e[:])
    if p == 0:
        nc.scalar.copy(ctxu[:, 1, :], ctx_o[:])
    else:
        nc.vector.tensor_copy(ctxu[:, 1, :], ctx_o[:])
    recip_sb = npool.tile([HEAD_DIM + 1, 2, CHUNK], F32R, tag="recip", bufs=2)
    with nc.allow_low_precision(reason="softmax denominators are O(1e3); fp32r's 11-bit mantissa is plenty"):
        nc.vector.reciprocal(recip_sb[64:65, :, :], ctxu[64:65, :, :])
    if p == 1:
        # caller emits latency-tolerant oproj matmuls before the bc chain
        return ctxu, recip_sb
    _norm_finish(nc, psum, npool, p, c, ctxu, recip_sb, ctxT_sb, ones64)
    return None


def _norm_finish(nc, psum, npool, p, c, ctxu, recip_sb, ctxT_sb, ones64):
    mm = nc.tensor.matmul
    bc_e = psum.tile([HEAD_DIM, CHUNK], F32, tag="ctx_e", bufs=1)
    bc_o = psum.tile([HEAD_DIM, CHUNK], F32, tag="ctx_o", bufs=1)
    for hl in range(2):
        mm(bc_e if hl == 0 else bc_o,
           ones64[64:65, :],
           recip_sb[64:65, hl, :],
           start=True, stop=True)
    if p == 1 and c == NCH - 1:
        # final chunk: the even half writes in place (partition-aligned);
        # the odd half skips the ~2.5µs bounce-DMA chain — the output
        # projection reads tmp_f directly as an extra K=64 step
        nc.vector.tensor_mul(ctxT_sb[0:64, p, c * CHUNK:(c + 1) * CHUNK],
                             ctxu[0:64, 0, :], bc_e[:])
        tmp_f = npool.tile([HEAD_DIM, CHUNK], F32R, tag="tmp_o", bufs=2)
        nc.vector.tensor_mul(tmp_f[:], ctxu[0:64, 1, :], bc_o[:])
        return tmp_f
    # odd-half mul first: its bounce DMA is on the next oproj quarter's
    # critical path, the even half's in-place write is not
    tmp_o = npool.tile([HEAD_DIM, CHUNK], F32R, tag="tmp_o", bufs=2)
    nc.vector.tensor_mul(tmp_o[:], ctxu[0:64, 1, :], bc_o[:])
    # partition-shifting bounce must be a DMA: engine lanes are
    # partition-locked, and GPSIMD cannot access PSUM on TRN2 anyway
    nc.sync.dma_start(out=ctxT_sb[64:128, p, c * CHUNK:(c + 1) * CHUNK],
                      in_=tmp_o[:])
    nc.vector.tensor_mul(ctxT_sb[0:64, p, c * CHUNK:(c + 1) * CHUNK],
                         ctxu[0:64, 0, :], bc_e[:])
    return tmp_o


def build_program(masked=False, with_bias=False):
    key = (masked, with_bias)
    if key in _PROGRAM_CACHE:
        return _PROGRAM_CACHE[key]
    nc = bacc.Bacc("TRN2", target_bir_lowering=False, debug=False,
                   enable_asserts=False)
    xT = nc.dram_tensor("xT", [H, S], BF16, kind="ExternalInput").ap()
    wq = nc.dram_tensor("wq", [H, HD], BF16, kind="ExternalInput").ap()
    wk = nc.dram_tensor("wk", [H, HD], BF16, kind="ExternalInput").ap()
    wv = nc.dram_tensor("wv", [H, HD], BF16, kind="ExternalInput").ap()
    wo = nc.dram_tensor("wo", [HD, H], F32R, kind="ExternalInput").ap()
    bq = nc.dram_tensor("bq", [1, HD], F32R, kind="ExternalInput").ap()
    bk = nc.dram_tensor("bk", [1, HD], F32R, kind="ExternalInput").ap()
    bv = nc.dram_tensor("bv", [1, HD], F32R, kind="ExternalInput").ap()
    am = nc.dram_tensor("am", [P, KT_S], F32, kind="ExternalInput").ap()
    o = nc.dram_tensor("o_part", [S, H], F32, kind="ExternalOutput").ap()
    with tile.TileContext(nc) as tc:
        _emit(tc, nc, (xT, wq, wk, wv, wo, bq, bk, bv, am, o), masked, with_bias)
    nc.compile()
    _PROGRAM_CACHE[key] = nc
    return nc


def _round_fp32r(a):
    """Round fp32 to the PE's fp32r format (11 mantissa bits, RNE)."""
    u = np.ascontiguousarray(a, np.float32).view(np.uint32)
    r = (u + np.uint32(0x7FF) + ((u >> np.uint32(12)) & np.uint32(1))) \
        & np.uint32(0xFFFFF000)
    return r.view(np.float32)


def make_in_maps(hidden_states, attention_mask, Wq, bq, Wk, bk, Wv, bv, Wo, bo):
    """Per-core input dicts. Core c: batch c//4, head-group c%4.

    Wq/bq are pre-scaled by 1/8 (= 1/sqrt(HEAD_DIM), exact in fp32) so the
    kernel's raw scores are already scaled. Tensors feeding float32r
    matmuls are pre-rounded to fp32r on the host (the device DMAs them
    into float32r tiles verbatim).
    """
    import ml_dtypes
    bf16 = ml_dtypes.bfloat16
    hidden_states = np.asarray(hidden_states, np.float32)
    attention_mask = np.asarray(attention_mask, np.float32)
    xTs = [np.ascontiguousarray(hidden_states[b].T).astype(bf16) for b in range(B)]
    ams = []
    for b in range(B):
        amask = ((1.0 - attention_mask[b]) * -10000.0).astype(np.float32)
        ams.append(np.ascontiguousarray(amask.reshape(KT_S, P).T))
    in_maps = []
    for c in range(N_CORES):
        b, g = divmod(c, GROUPS)
        hs = slice(g * HD, (g + 1) * HD)
        in_maps.append({
            "xT": xTs[b],
            "wq": (np.asarray(Wq, np.float32)[hs, :].T * np.float32(0.125)).astype(bf16),
            "wk": np.asarray(Wk, np.float32)[hs, :].T.astype(bf16),
            "wv": np.asarray(Wv, np.float32)[hs, :].T.astype(bf16),
            "wo": _round_fp32r(np.asarray(Wo, np.float32)[:, hs].T),
            "bq": _round_fp32r(np.asarray(bq, np.float32)[hs].reshape(1, HD) * np.float32(0.125)),
            "bk": _round_fp32r(np.asarray(bk, np.float32)[hs].reshape(1, HD)),
            "bv": _round_fp32r(np.asarray(bv, np.float32)[hs].reshape(1, HD)),
            "am": ams[b],
        })
    return in_maps


def kernel(hidden_states, attention_mask, Wq, bq, Wk, bk, Wv, bv, Wo, bo):
    masked = not bool(np.all(np.asarray(attention_mask) == 1.0))
    with_bias = not (np.all(np.asarray(bq) == 0) and np.all(np.asarray(bk) == 0)
                     and np.all(np.asarray(bv) == 0))
    nc = build_program(masked, with_bias)
    in_maps = make_in_maps(hidden_states, attention_mask,
                           Wq, bq, Wk, bk, Wv, bv, Wo, bo)
    res = run_bass_kernel_spmd(nc, in_maps, core_ids=list(range(N_CORES)))
    out = np.zeros((B, S, H), np.float32)
    for c in range(N_CORES):
        b = c // GROUPS
        out[b] += res.results[c]["o_part"]
    out += np.asarray(bo, np.float32)
    return out



# revision 55
# speedup vs baseline: 1.0030x; 1.0029x over previous
"""Trainium2 Bass kernel: 16-head attention block (B=2, S=2048, H=1024).

Sharding: 8 cores = 2-way data parallel (batch) x 4-way tensor parallel
(head groups of 4 heads / 256 dims). Each core computes, for its batch
and head group:
    Q^T, K^T (= W @ x^T, [dims, seq] layout; Wq/bq pre-scaled by 1/8 on
    host so no score scaling is needed on device), V ([seq, dims]),
    S^T = K Q^T per head (key positions on partitions),
    P^T = exp(S^T + mask),
    ctx'^T = [V | 1]^T P^T    (ones column folded in -> row 64 = softmax
                               denominator),
    ctx^T normalized, then partial output O_g = ctx^T.T @ Wo[:,hs]^T.
Host sums the 4 partial outputs per batch and adds bo.

Precision: x and the QKV weights ship as bf16 (halves the inbound DMA);
scores run as float32r; the probabilities P and V are bf16 for the ctx
matmul; the output projection is float32r.  3/8 of the softmax exp
half-tiles run as an int16 Schraudolph (bitcast) on the otherwise-
saturated DVE instead of the ACT engine.  Measured end-to-end rel err
~1.0e-2 against the fp32 reference (gate: 2e-2).

Overlap: emission order interleaves pair-1 projections under pair-0's
softmax shadow and the output projection under pair-1's; per-head score
psum tiles + a 6-deep pt pool keep the PE's score->exp->ctx pipeline
from stalling on any single engine's queue.
"""

import contextlib

import numpy as np

import concourse.bass as bass
import concourse.mybir as mybir
import concourse.tile as tile
from concourse import bacc
from concourse.bass_utils import run_bass_kernel_spmd

B, S, H = 2, 2048, 1024
NUM_HEADS, HEAD_DIM = 16, 64
N_CORES = 8
GROUPS = 4                  # head-parallel groups per batch
HD = H // GROUPS            # 256 head-dims per core (4 heads)
P = 128
KT_H = H // P               # 8 k-tiles over hidden dim
KT_S = S // P               # 16 k-tiles over sequence (key positions)
NCH = 4                     # q chunks
CHUNK = S // NCH            # 512
F32 = mybir.dt.float32
F32R = mybir.dt.float32r
BF16 = mybir.dt.bfloat16
I16 = mybir.dt.int16
EXP = mybir.ActivationFunctionType.Exp

# exp(s) ~= bitcast_bf16(int16(s * 2^7/ln2 + (127*2^7 - c))).  c tuned for
# min-rms relative ripple of the linear-mantissa approximation (~1.5% rms).
# The DVE/Pool f32->i16 convert is round-to-nearest (verified on device);
# softmax's ratio cancels the uniform component, leaving only the ripple.
SCH_A = float(128.0 / np.log(2.0))
SCH_B = float(127.0 * 128.0 - 0.043677448 * 128.0)
# Per-(kt, head-half) exp engine: 'A' = ACT (exact exp), 'V' = DVE
# (int16 Schraudolph).  The Pool engine cannot touch PSUM on TRN2, so it
# only carries SBUF->SBUF traffic (the ctxT bounce copies).  ACT keeps
# 20/32 half-tiles; the DVE share is sized so neither queue bursts past
# the PE's per-kt pace.
def _exp_sched(p):
    sched = {}
    dve_o = (1, 3, 5, 7, 9, 11, 13, 15)
    dve_e = (2, 6, 10, 14)
    for kt in range(KT_S):
        sched[(kt, 0)] = 'V' if kt in dve_e else 'A'
        sched[(kt, 1)] = 'V' if kt in dve_o else 'A'
    return sched


EXP_SCHED = (_exp_sched(0), _exp_sched(1))

_PROGRAM_CACHE = {}


def _emit(tc, nc, dram, masked, with_bias):
    mm = nc.tensor.matmul
    xT_d, wq_d, wk_d, wv_d, wo_d, bq_d, bk_d, bv_d, am_d, o_d = dram

    stack = contextlib.ExitStack()
    with stack:
        const = stack.enter_context(tc.tile_pool(name="const", bufs=1))
        big = stack.enter_context(tc.tile_pool(name="big", bufs=1))

        onesf = const.tile([P, 64], F32)
        nc.any.memset(onesf[:], 1.0)
        ones64 = const.tile([P, 64], F32R)   # lane-64 row used as K=1 lhsT
        nc.vector.tensor_copy(ones64[:], onesf[:])
        onesb = const.tile([P, 64], BF16)
        nc.vector.tensor_copy(onesb[:], onesf[:])
        # warm the ACT exp table before it is first needed
        trash = const.tile([1, 16], F32)
        nc.scalar.activation(trash[:], onesf[0:1, 0:16], EXP)
        if masked:
            amask_sb = const.tile([P, KT_S], F32)
            nc.sync.dma_start(out=amask_sb[:], in_=am_d[:])
        if with_bias:
            ones_sb = const.tile([1, 512], F32R)
            for i in range(8):
                nc.vector.tensor_copy(ones_sb[0:1, i * 64:(i + 1) * 64],
                                      onesf[0:1, :])
            bq_sb = const.tile([1, HD], F32R)
            nc.sync.dma_start(out=bq_sb[:], in_=bq_d[:])
            bk_sb = const.tile([1, HD], F32R)
            nc.sync.dma_start(out=bk_sb[:], in_=bk_d[:])
            bv_sb = const.tile([1, HD], F32R)
            nc.sync.dma_start(out=bv_sb[:], in_=bv_d[:])
        wo_sb = const.tile([P, HD // P, H], F32R)

        # persistent activations
        qT_sb = big.tile([P, 2, S], F32R)    # [dim-in-pair, pair, seq]
        kT_sb = big.tile([P, 2, S], F32R)
        v_sb = big.tile([P, KT_S, GROUPS, HEAD_DIM + 1], BF16)  # [seq, kt, head, d+1]
        ctxT_sb = big.tile([P, 2, S], F32R)

        # ones column of V' (the rowsum trick)
        nc.vector.tensor_copy(v_sb[:, :, :, HEAD_DIM:HEAD_DIM + 1], onesb[:, 0:KT_S * GROUPS])

        # ---------- input DMAs, interleaved k-tile-wise so the pair-0 K/Q
        # projection for seq-chunk 0 can start ~0.5MB into the inbound
        # stream instead of after all of wk+wq (4µs vs 14µs) ----------
        kq_pool = tc.alloc_tile_pool(name="kq_pool", bufs=1, side="right")
        wv_stack = contextlib.ExitStack()
        wv_pool = wv_stack.enter_context(tc.tile_pool(name="wv_pool", bufs=1, side="right"))
        wk_sb = kq_pool.tile([P, KT_H, HD], BF16)
        xT_sb = kq_pool.tile([P, KT_H, S], BF16)
        wq_sb = kq_pool.tile([P, KT_H, HD], BF16)
        wv_sb = wv_pool.tile([P, KT_H, HD], BF16)
        # one batched DMA per tensor (per seq-chunk for x): issuing from the
        # SP queue costs ~565ns per dma_start, so 24 small DMAs would pace
        # the whole projection head
        wk_r = wk_d.rearrange("(t p) c -> p t c", p=P)
        xT_r = xT_d.rearrange("(t p) s -> p t s", p=P)
        # chunk 0 and wk arrive in k-tile pieces, wq between them: the K
        # projection needs only wk + x, so the first matmuls start ~3µs in
        nc.sync.dma_start(out=wk_sb[:, 0:4, :], in_=wk_r[:, 0:4, :])
        nc.sync.dma_start(out=xT_sb[:, 0:2, 0:512], in_=xT_r[:, 0:2, 0:512])
        nc.sync.dma_start(out=xT_sb[:, 2:4, 0:512], in_=xT_r[:, 2:4, 0:512])
        nc.sync.dma_start(out=wk_sb[:, 4:8, :], in_=wk_r[:, 4:8, :])
        nc.sync.dma_start(out=wq_sb[:], in_=wq_d.rearrange("(t p) c -> p t c", p=P))
        nc.sync.dma_start(out=xT_sb[:, 4:8, 0:512], in_=xT_r[:, 4:8, 0:512])
        for cc in range(1, 4):
            nc.sync.dma_start(out=xT_sb[:, :, cc * 512:(cc + 1) * 512],
                              in_=xT_r[:, :, cc * 512:(cc + 1) * 512])
        nc.sync.dma_start(out=wv_sb[:], in_=wv_d.rearrange("(t p) c -> p t c", p=P))

        # all projection psum traffic lives in two persistent banks (tags
        # ps_k / ps_q, alternating for double-buffering) so the attention
        # pool can hold its six banks for the whole kernel with no
        # pool-boundary barrier between projections and attention
        def qk_proj(ps_pool, w_sb, b_sb, dst, p):
            for c4 in range(4):
                ps_qk = ps_pool.tile([P, 512], F32, bufs=1, name="ps_qk",
                                     tag="ps_k" if c4 % 2 == 0 else "ps_q")
                for kt in range(KT_H):
                    mm(ps_qk[:],
                       w_sb[:, kt, p * P:(p + 1) * P],
                       xT_sb[:, kt, c4 * 512:(c4 + 1) * 512],
                       start=(kt == 0), stop=(not with_bias and kt == KT_H - 1))
                if with_bias:
                    mm(ps_qk[:],
                       b_sb[:, p * P:(p + 1) * P],
                       ones_sb[:, 0:512],
                       start=False, stop=True)
                nc.vector.tensor_copy(dst[:, p, c4 * 512:(c4 + 1) * 512], ps_qk[:])

        def oproj_quarter(o_ps, opool, q, pre=None, tmp_f=None):
            # psum->sbuf copies split across ACT and DVE; each half-tile
            # DMAs out as soon as it is copied.  For the final quarter the
            # pair-1 odd ctx half never went through the bounce DMA: it is
            # read from tmp_f (partitions 0-63) as an extra K=64 step, and
            # the first m-tile's pair-0 matmuls were pre-emitted (`pre`)
            # ahead of the norm chain so the PE keeps streaming through it.
            final = (q == NCH - 1)
            for m in range(4 * q, 4 * q + 4):
                o_sb = opool.tile([P, H], F32, tag="o_sb", bufs=3)
                for n2 in range(2):
                    ns = slice(n2 * 512, (n2 + 1) * 512)
                    if pre is not None and m == 4 * q:
                        ps_o = pre[n2]
                    else:
                        ps_o = o_ps.tile([P, 512], F32, tag="ps_o", bufs=2)
                        mm(ps_o[:], ctxT_sb[:, 0, m * P:(m + 1) * P],
                           wo_sb[:, 0, ns], start=True, stop=False)
                    if final:
                        # read the odd ctx half straight from tmp_f — the
                        # final quarter must not wait on the bounce DMA
                        mm(ps_o[:], ctxT_sb[0:64, 1, m * P:(m + 1) * P],
                           wo_sb[0:64, 1, ns], start=False, stop=False)
                        mc = (m - 4 * q) * P
                        mm(ps_o[:], tmp_f[:, mc:mc + P],
                           wo_odd[:, ns], start=False, stop=True)
                    else:
                        mm(ps_o[:], ctxT_sb[:, 1, m * P:(m + 1) * P],
                           wo_sb[:, 1, ns], start=False, stop=True)
                    if n2 == 0:
                        nc.scalar.copy(o_sb[:, 0:512], ps_o[:])
                    else:
                        nc.vector.tensor_copy(o_sb[:, 512:1024], ps_o[:])
                    nc.sync.dma_start(
                        out=o_d[m * P:(m + 1) * P, ns],
                        in_=o_sb[:, ns])

        def v_proj(ms, pool):
            for m in ms:
                ps_v = pool.tile([P, HD], F32, bufs=1, name="ps_v",
                                 tag="ps_k" if m % 2 == 0 else "ps_q")
                for kt in range(KT_H):
                    mm(ps_v[:],
                       xT_sb[:, kt, m * P:(m + 1) * P],
                       wv_sb[:, kt, :],
                       start=(kt == 0), stop=(not with_bias and kt == KT_H - 1))
                if with_bias:
                    mm(ps_v[:],
                       ones_sb[:, 0:P],
                       bv_sb[:],
                       start=False, stop=True)
                nc.vector.tensor_copy(v_sb[:, m, :, 0:HEAD_DIM], ps_v[:])

        # ---------- pools: attention psum first (banks 0-5, alive for the
        # whole kernel), projections in the remaining two banks ----------
        attn_stack = contextlib.ExitStack()
        a_ps = attn_stack.enter_context(
            tc.tile_pool(name="attn_psum", bufs=1, space="PSUM"))
        ptp = attn_stack.enter_context(
            tc.tile_pool(name="pt_pool", bufs=6 if not (masked or with_bias) else 2))
        npool = attn_stack.enter_context(tc.tile_pool(name="norm_pool", bufs=2))
        psA = tc.alloc_tile_pool(name="proj_psum", bufs=1, space="PSUM")

        # pair-0 K/Q projections, seq-chunk-outer so each inbound x
        # column-chunk is consumed as soon as it lands; score tiles for
        # seq-chunk 0 can then start ~3MB into the inbound DMA
        for cc in range(4):
            ps_k = psA.tile([P, 512], F32, tag="ps_k", bufs=1, name="ps_k")
            ps_q = psA.tile([P, 512], F32, tag="ps_q", bufs=1, name="ps_q")
            # chunk 0 runs K fully before Q (K needs only wk + x; wq is
            # still inbound); later chunks interleave for psum parallelism
            pairs = (((ps_k, wk_sb),), ((ps_q, wq_sb),)) if cc == 0 \
                else (((ps_k, wk_sb), (ps_q, wq_sb)),)
            for group in pairs:
                for kt in range(KT_H):
                    for ps, w_sb in group:
                        mm(ps[:],
                           w_sb[:, kt, 0:P],
                           xT_sb[:, kt, cc * 512:(cc + 1) * 512],
                           start=(kt == 0),
                           stop=(not with_bias and kt == KT_H - 1))
            if with_bias:
                for ps, b_sb in ((ps_k, bk_sb), (ps_q, bq_sb)):
                    mm(ps[:],
                       b_sb[:, 0:P],
                       ones_sb[:, 0:512],
                       start=False, stop=True)
            nc.vector.tensor_copy(kT_sb[:, 0, cc * 512:(cc + 1) * 512], ps_k[:])
            nc.vector.tensor_copy(qT_sb[:, 0, cc * 512:(cc + 1) * 512], ps_q[:])

        v_proj(range(KT_S), psA)

        # pair-0 attention; V projection + pair-1 projections fill the PE
        # shadow under the ACT-bound softmax
        for c in range(NCH):
            _attn_one_chunk(tc, nc, a_ps, ptp, npool, 0, c, masked,
                            amask_sb if masked else None,
                            kT_sb, qT_sb, v_sb, ctxT_sb, ones64)
        qk_proj(psA, wk_sb, bk_sb if with_bias else None, kT_sb, 1)
        qk_proj(psA, wq_sb, bq_sb if with_bias else None, qT_sb, 1)
        wv_stack.close()

        # wo arrives during pair-0 attention; needed only in the final phase
        nc.sync.dma_start(out=wo_sb[:], in_=wo_d.rearrange("(t p) c -> p t c", p=P))
        # pair-1 odd-head rows of wo, lowered to partitions 0-63: lets the
        # final chunk's output projection read the un-bounced odd ctx (which
        # lives at partitions 0-63) as a separate K=64 accumulation step
        wo_odd = const.tile([64, H], F32R)
        nc.sync.dma_start(out=wo_odd[:], in_=wo_sb[64:128, 1, :])
        kq_pool.release()
        psA.release()

        # pair-1 attention with the output projection interleaved per chunk
        o_ps = attn_stack.enter_context(tc.tile_pool(name="o_psum", bufs=1, space="PSUM"))
        opool = attn_stack.enter_context(tc.tile_pool(name="o_pool", bufs=1))
        for c in range(NCH):
            ctxu1, recip1 = _attn_one_chunk(
                tc, nc, a_ps, ptp, npool, 1, c, masked,
                amask_sb if masked else None,
                kT_sb, qT_sb, v_sb, ctxT_sb, ones64)
            # slot the first m-tile's pair-0 matmuls in front of the norm
            # chain (bc/mul/bounce) the PE would otherwise wait on
            pre = []
            for n2 in range(2):
                ps_p = o_ps.tile([P, 512], F32, tag="ps_o", bufs=2,
                                 name="ps_p")
                mm(ps_p[:], ctxT_sb[:, 0, 4 * c * P:(4 * c + 1) * P],
                   wo_sb[:, 0, n2 * 512:(n2 + 1) * 512],
                   start=True, stop=False)
                pre.append(ps_p)
            tmp_f = _norm_finish(nc, a_ps, npool, 1, c, ctxu1, recip1,
                                 ctxT_sb, ones64)
            oproj_quarter(o_ps, opool, c, pre, tmp_f)
        attn_stack.close()


def _attn_one_chunk(tc, nc, psum, ptp, npool, p, c, masked, amask_sb,
                    kT_sb, qT_sb, v_sb, ctxT_sb, ones64):
    mm = nc.tensor.matmul
    ctx_e = psum.tile([HEAD_DIM + 1, CHUNK], F32, tag="ctx_e", bufs=1)
    ctx_o = psum.tile([HEAD_DIM + 1, CHUNK], F32, tag="ctx_o", bufs=1)
    sched = EXP_SCHED[p]
    for kt in range(KT_S):
        # separate per-head score tiles: each half releases to the next
        # score matmul as soon as ITS exp drains, halving pipeline stalls
        s_half = [psum.tile([P, CHUNK], F32, tag=f"s_{hl}", bufs=2, name=f"s_{hl}")
                  for hl in range(2)]
        for hl in range(2):
            mm(s_half[hl][:],
               kT_sb[hl * 64:(hl + 1) * 64, p, kt * P:(kt + 1) * P],
               qT_sb[hl * 64:(hl + 1) * 64, p, c * CHUNK:(c + 1) * CHUNK],
               start=True, stop=True)
        pt = ptp.tile([P, 2 * CHUNK], BF16, tag="pt")
        for hl in range(2):
            dst = pt[:, hl * CHUNK:(hl + 1) * CHUNK]
            if masked:
                nc.scalar.activation(dst, s_half[hl][:], EXP,
                                     bias=amask_sb[:, kt:kt + 1])
            elif sched[(kt, hl)] == 'A':
                nc.scalar.activation(dst, s_half[hl][:], EXP)
            else:
                # softmax exp off the ACT engine: bitcast-int16 Schraudolph
                # (~1.5% rms ripple; softmax's ratio cancels the mean shift)
                eng = nc.vector if sched[(kt, hl)] == 'V' else nc.gpsimd
                eng.tensor_scalar(dst.bitcast(I16), s_half[hl][:], SCH_A, SCH_B,
                                  mybir.AluOpType.mult, mybir.AluOpType.add)
        for hl in range(2):
            ctx = ctx_e if hl == 0 else ctx_o
            mm(ctx[:],
               v_sb[:, kt, 2 * p + hl, :],
               pt[:, hl * CHUNK:(hl + 1) * CHUNK],
               start=(kt == 0), stop=(kt == KT_S - 1))
    # chunk-end psum drain off the DVE (a DVE burst here head-of-line-blocks
    # the next chunk's exp tiles); pair-1 splits it ACT/Pool to halve either
    # queue's burst
    ctxu = npool.tile([HEAD_DIM + 1, 2, CHUNK], F32, tag="ctxu", bufs=2)
    nc.scalar.copy(ctxu[:, 0, :], ctx_e[:])
    if p == 0:
        nc.scalar.copy(ctxu[:, 1, :], ctx_o[:])
    else:
        nc.vector.tensor_copy(ctxu[:, 1, :], ctx_o[:])
    recip_sb = npool.tile([HEAD_DIM + 1, 2, CHUNK], F32R, tag="recip", bufs=2)
    with nc.allow_low_precision(reason="softmax denominators are O(1e3); fp32r's 11-bit mantissa is plenty"):
        if p == 1 and c == NCH - 1:
            # split so bc_e can issue off recip_e alone — this chain is the
            # kernel's tail
            nc.vector.reciprocal(recip_sb[64:65, 0, :], ctxu[64:65, 0, :])
            nc.vector.reciprocal(recip_sb[64:65, 1, :], ctxu[64:65, 1, :])
        else:
            nc.vector.reciprocal(recip_sb[64:65, :, :], ctxu[64:65, :, :])
    if p == 1:
        # caller emits latency-tolerant oproj matmuls before the bc chain
        return ctxu, recip_sb
    _norm_finish(nc, psum, npool, p, c, ctxu, recip_sb, ctxT_sb, ones64)
    return None


def _norm_finish(nc, psum, npool, p, c, ctxu, recip_sb, ctxT_sb, ones64):
    mm = nc.tensor.matmul
    bc_e = psum.tile([HEAD_DIM, CHUNK], F32, tag="ctx_e", bufs=1)
    bc_o = psum.tile([HEAD_DIM, CHUNK], F32, tag="ctx_o", bufs=1)
    for hl in range(2):
        mm(bc_e if hl == 0 else bc_o,
           ones64[64:65, :],
           recip_sb[64:65, hl, :],
           start=True, stop=True)
    if p == 1 and c == NCH - 1:
        # final chunk: the even half writes in place (partition-aligned);
        # the odd half skips the ~2.5µs bounce-DMA chain — the output
        # projection reads tmp_f directly as an extra K=64 step
        nc.vector.tensor_mul(ctxT_sb[0:64, p, c * CHUNK:(c + 1) * CHUNK],
                             ctxu[0:64, 0, :], bc_e[:])
        tmp_f = npool.tile([HEAD_DIM, CHUNK], F32R, tag="tmp_o", bufs=2)
        nc.vector.tensor_mul(tmp_f[:], ctxu[0:64, 1, :], bc_o[:])
        return tmp_f
    # odd-half mul first: its bounce DMA is on the next oproj quarter's
    # critical path, the even half's in-place write is not
    tmp_o = npool.tile([HEAD_DIM, CHUNK], F32R, tag="tmp_o", bufs=2)
    nc.vector.tensor_mul(tmp_o[:], ctxu[0:64, 1, :], bc_o[:])
    # partition-shifting bounce must be a DMA: engine lanes are
    # partition-locked, and GPSIMD cannot access PSUM on TRN2 anyway
    nc.sync.dma_start(out=ctxT_sb[64:128, p, c * CHUNK:(c + 1) * CHUNK],
                      in_=tmp_o[:])
    nc.vector.tensor_mul(ctxT_sb[0:64, p, c * CHUNK:(c + 1) * CHUNK],
                         ctxu[0:64, 0, :], bc_e[:])
    return tmp_o


def build_program(masked=False, with_bias=False):
    key = (masked, with_bias)
    if key in _PROGRAM_CACHE:
        return _PROGRAM_CACHE[key]
    nc = bacc.Bacc("TRN2", target_bir_lowering=False, debug=False,
                   enable_asserts=False)
    xT = nc.dram_tensor("xT", [H, S], BF16, kind="ExternalInput").ap()
    wq = nc.dram_tensor("wq", [H, HD], BF16, kind="ExternalInput").ap()
    wk = nc.dram_tensor("wk", [H, HD], BF16, kind="ExternalInput").ap()
    wv = nc.dram_tensor("wv", [H, HD], BF16, kind="ExternalInput").ap()
    wo = nc.dram_tensor("wo", [HD, H], F32R, kind="ExternalInput").ap()
    bq = nc.dram_tensor("bq", [1, HD], F32R, kind="ExternalInput").ap()
    bk = nc.dram_tensor("bk", [1, HD], F32R, kind="ExternalInput").ap()
    bv = nc.dram_tensor("bv", [1, HD], F32R, kind="ExternalInput").ap()
    am = nc.dram_tensor("am", [P, KT_S], F32, kind="ExternalInput").ap()
    o = nc.dram_tensor("o_part", [S, H], F32, kind="ExternalOutput").ap()
    with tile.TileContext(nc) as tc:
        _emit(tc, nc, (xT, wq, wk, wv, wo, bq, bk, bv, am, o), masked, with_bias)
    nc.compile()
    _PROGRAM_CACHE[key] = nc
    return nc


def _round_fp32r(a):
    """Round fp32 to the PE's fp32r format (11 mantissa bits, RNE)."""
    u = np.ascontiguousarray(a, np.float32).view(np.uint32)
    r = (u + np.uint32(0x7FF) + ((u >> np.uint32(12)) & np.uint32(1))) \
        & np.uint32(0xFFFFF000)
    return r.view(np.float32)


def make_in_maps(hidden_states, attention_mask, Wq, bq, Wk, bk, Wv, bv, Wo, bo):
    """Per-core input dicts. Core c: batch c//4, head-group c%4.

    Wq/bq are pre-scaled by 1/8 (= 1/sqrt(HEAD_DIM), exact in fp32) so the
    kernel's raw scores are already scaled. Tensors feeding float32r
    matmuls are pre-rounded to fp32r on the host (the device DMAs them
    into float32r tiles verbatim).
    """
    import ml_dtypes
    bf16 = ml_dtypes.bfloat16
    hidden_states = np.asarray(hidden_states, np.float32)
    attention_mask = np.asarray(attention_mask, np.float32)
    xTs = [np.ascontiguousarray(hidden_states[b].T).astype(bf16) for b in range(B)]
    ams = []
    for b in range(B):
        amask = ((1.0 - attention_mask[b]) * -10000.0).astype(np.float32)
        ams.append(np.ascontiguousarray(amask.reshape(KT_S, P).T))
    in_maps = []
    for c in range(N_CORES):
        b, g = divmod(c, GROUPS)
        hs = slice(g * HD, (g + 1) * HD)
        in_maps.append({
            "xT": xTs[b],
            "wq": (np.asarray(Wq, np.float32)[hs, :].T * np.float32(0.125)).astype(bf16),
            "wk": np.asarray(Wk, np.float32)[hs, :].T.astype(bf16),
            "wv": np.asarray(Wv, np.float32)[hs, :].T.astype(bf16),
            "wo": _round_fp32r(np.asarray(Wo, np.float32)[:, hs].T),
            "bq": _round_fp32r(np.asarray(bq, np.float32)[hs].reshape(1, HD) * np.float32(0.125)),
            "bk": _round_fp32r(np.asarray(bk, np.float32)[hs].reshape(1, HD)),
            "bv": _round_fp32r(np.asarray(bv, np.float32)[hs].reshape(1, HD)),
            "am": ams[b],
        })
    return in_maps


def kernel(hidden_states, attention_mask, Wq, bq, Wk, bk, Wv, bv, Wo, bo):
    masked = not bool(np.all(np.asarray(attention_mask) == 1.0))
    with_bias = not (np.all(np.asarray(bq) == 0) and np.all(np.asarray(bk) == 0)
                     and np.all(np.asarray(bv) == 0))
    nc = build_program(masked, with_bias)
    in_maps = make_in_maps(hidden_states, attention_mask,
                           Wq, bq, Wk, bk, Wv, bv, Wo, bo)
    res = run_bass_kernel_spmd(nc, in_maps, core_ids=list(range(N_CORES)))
    out = np.zeros((B, S, H), np.float32)
    for c in range(N_CORES):
        b = c // GROUPS
        out[b] += res.results[c]["o_part"]
    out += np.asarray(bo, np.float32)
    return out

